# revision 29
# baseline (speedup 1.0000x reference)
"""Expert-parallel MoE classifier kernel for 8 Trainium2 NeuronCores.

Strategy
--------
The reference runs every expert densely over the whole batch, but the output
only uses each token's top-2 experts.  We therefore:

  1. (host) run the tiny router in fp32 numpy: logits -> softmax -> top-2 ->
     renormalized combine weights.  This exactly reproduces the jax routing
     (verified: identical top-2 indices, weight diff ~2e-6).
  2. (host) dispatch: gather each expert's assigned tokens into 128-token
     blocks; assign blocks round-robin to the 8 cores (expert-parallel
     sharding of proj_w/layer_w/cls_w: each expert's weights go only to the
     core that owns its block(s)).
  3. (device, SPMD on 8 cores) each core runs its NB blocks through the
     expert MLP stack: proj(267->1024) + gelu, 4x hidden(1024->1024) + gelu,
     cls(1024->5).  bf16 matmuls with fp32 PSUM accumulation; weights are
     the stationary operand, tokens ride the moving free dim.
  4. (host) combine: out[b] = w0*o_{e0}[b] + w1*o_{e1}[b] (+ cls bias).

Everything heavy (99.9% of FLOPs) runs on the NeuronCores.
"""

import sys
import types

import numpy as np
import ml_dtypes

import concourse.bacc as bacc
import concourse.tile as tile
import concourse.mybir as mybir
from concourse.bass_utils import run_bass_kernel_spmd


def _ensure_ntff_hook():
    """Provide antenv.axon_hooks if the image's antenv lacks it, so
    trace=True NTFF profiling works under axon (see trn_agent_boot)."""
    try:
        from antenv.axon_hooks import get_axon_ntff_profile_hook  # noqa: F401
        return
    except ImportError:
        pass
    try:
        from trn_agent_boot.trn_boot import _ntff_profile_via_ctypes
        hook = _ntff_profile_via_ctypes("/opt/axon/libaxon_pjrt.so")
    except Exception:
        hook = None
    mod = types.ModuleType("antenv.axon_hooks")
    mod.get_axon_ntff_profile_hook = lambda: hook
    mod.set_axon_ntff_profile_hook = lambda h: None
    sys.modules["antenv.axon_hooks"] = mod


_ensure_ntff_hook()

# Model shape (hardcoded per problem spec)
N_EXPERTS = 23
IN_DIM = 267
HID = 1024
N_LAYERS = 4
N_CLASSES = 5
TOP_K = 2
BATCH = 1024
N_CORES = 8

C = 128          # tokens per block (moving free dim of every matmul).
                 # Sized so PE compute per layer (~64 matmuls of N=C) fills
                 # the ~4.8us layer weight-DMA period: the PE never idles
                 # long enough to trip the HAM clock throttle. The extra
                 # token padding is free -- the DMA stream is the floor.
KP = 3           # K-tiles for the 267-dim input contraction (padded to 384)
MT = HID // 128  # 8 m-tiles (output partition tiles) per 1024-wide layer
MG = MT // 2     # m-tile pairs: two m-tiles share one PSUM bank / one ACT
CLS_PAD = 8      # classes padded 5 -> 8

BF16 = ml_dtypes.bfloat16

_PROGRAM_CACHE = {}


# merged per-block "smalls" tensor column offsets (bf16 columns per partition):
#   [0:3C)         x.T      3 K-tiles  (rows 267..383 zero)
#   [3C:3C+3072)   proj_w.T 3 K-tiles x 1024
#   [+64]          cls_w.T  8 K-tiles x 8
SM_X = 0
SM_PW = KP * C
SM_CW = SM_PW + KP * HID
SMW = SM_CW + MT * CLS_PAD


def _build_program(nb, use_bias):
    """Bass/Tile program for one core: nb expert blocks through the MLP."""
    dt = mybir.dt
    nc = bacc.Bacc("TRN2", target_bir_lowering=False, debug=False)

    sm_d = nc.dram_tensor("sm", [nb, 128, SMW], dt.bfloat16, kind="ExternalInput").ap()
    pb_d = nc.dram_tensor("pb", [nb, 128, MT], dt.float32, kind="ExternalInput").ap()
    # layer weights: [l, half, p, k, 512] (half = output-column half)
    lw_d = nc.dram_tensor("lw", [nb, N_LAYERS, 2, 128, MT, 512], dt.bfloat16, kind="ExternalInput").ap()
    lb_d = nc.dram_tensor("lb", [nb, 128, N_LAYERS, MT], dt.float32, kind="ExternalInput").ap()
    out_d = nc.dram_tensor("out", [nb, CLS_PAD, C], dt.float32, kind="ExternalOutput").ap()

    gelu = mybir.ActivationFunctionType.Gelu_apprx_tanh

    def act(hm, ps, bias_slices):
        # hm/ps: [128, 2, C]; bias_slices: list of (half, bias_ap[128,1]) or falsy
        if use_bias:
            for h2, b_ap in bias_slices:
                nc.scalar.activation(hm[:, h2, :], ps[:, h2, :], gelu, bias=b_ap)
        else:
            nc.scalar.activation(hm[:, :, :], ps[:, :, :], gelu)

    with tile.TileContext(nc) as tc:
        with (
            tc.tile_pool(name="wpool", bufs=9) as wpool,
            tc.tile_pool(name="small", bufs=max(3, nb)) as small,
            tc.tile_pool(name="hpool", bufs=4 * nb + 8) as hpool,
            tc.tile_pool(name="opool", bufs=2) as opool,
            tc.tile_pool(name="pspool", bufs=6, space="PSUM") as pspool,
            tc.tile_pool(name="cpspool", bufs=2, space="PSUM") as cpspool,
        ):
            smts = [None] * nb
            biases = [None] * nb
            hs = [None] * nb

            def emit_smt_dma(b):
                smt = small.tile([128, SMW], dt.bfloat16)
                nc.sync.dma_start(out=smt, in_=sm_d[b])
                smts[b] = smt
                if use_bias:
                    pbt = small.tile([128, MT], dt.float32)
                    nc.sync.dma_start(out=pbt, in_=pb_d[b])
                    lbt = small.tile([128, N_LAYERS, MT], dt.float32)
                    nc.sync.dma_start(out=lbt, in_=lb_d[b])
                    biases[b] = (pbt, lbt)

            def emit_proj(b):
                smt = smts[b]
                h = []
                for mg in range(MG):
                    ps = pspool.tile([128, 2, C], dt.float32)
                    for h2 in range(2):
                        m = 2 * mg + h2
                        for k in range(KP):
                            nc.tensor.matmul(
                                ps[:, h2, :],
                                smt[:, SM_PW + k * HID + m * 128: SM_PW + k * HID + (m + 1) * 128],
                                smt[:, SM_X + k * C: SM_X + (k + 1) * C],
                                start=(k == 0), stop=(k == KP - 1),
                            )
                    hm = hpool.tile([128, 2, C], dt.bfloat16)
                    act(hm, ps, use_bias and [(h2, biases[b][0][:, 2 * mg + h2:2 * mg + h2 + 1]) for h2 in range(2)])
                    h.append(hm)
                hs[b] = h

            def emit_layer(b, l):
                h = hs[b]
                lwt = wpool.tile([128, 2, MT, 512], dt.bfloat16)
                if b == nb - 1 and l == N_LAYERS - 1:
                    # final chunk of the stream in halves: tail compute
                    # overlaps the end of the stream
                    nc.sync.dma_start(out=lwt[:, 0], in_=lw_d[b, l, 0])
                    nc.sync.dma_start(out=lwt[:, 1], in_=lw_d[b, l, 1])
                else:
                    nc.sync.dma_start(
                        out=lwt, in_=lw_d[b, l].rearrange("two p k m -> p two k m")
                    )
                hn = []
                for mg in range(MG):
                    ps = pspool.tile([128, 2, C], dt.float32)
                    for h2 in range(2):
                        m = 2 * mg + h2
                        mcol = (m % 4) * 128
                        for k in range(MT):
                            nc.tensor.matmul(
                                ps[:, h2, :], lwt[:, m // 4, k, mcol:mcol + 128],
                                h[k // 2][:, k % 2, :],
                                start=(k == 0), stop=(k == MT - 1),
                            )
                    hm = hpool.tile([128, 2, C], dt.bfloat16)
                    act(hm, ps, use_bias and [(h2, biases[b][1][:, l, 2 * mg + h2:2 * mg + h2 + 1]) for h2 in range(2)])
                    hn.append(hm)
                hs[b] = hn

            def emit_cls(b):
                smt, h = smts[b], hs[b]
                cps = cpspool.tile([CLS_PAD, C], dt.float32)
                for k in range(MT):
                    nc.tensor.matmul(
                        cps[:], smt[:, SM_CW + k * CLS_PAD: SM_CW + (k + 1) * CLS_PAD],
                        h[k // 2][:, k % 2, :],
                        start=(k == 0), stop=(k == MT - 1),
                    )
                ot = opool.tile([CLS_PAD, C], dt.float32)
                nc.vector.tensor_copy(ot[:], cps[:])
                # output DMA rides the ACT HWDGE ring (tiny, and by dispatch
                # time its data is ready, so it doesn't stall weight DMAs)
                nc.scalar.dma_start(out=out_d[b], in_=ot[:])

            # Interleave the blocks layer-by-layer (A0,B0,C0,A1,B1,C1,...):
            # each layer's weight wait shrinks below the HAM idle window, so
            # the PE stays warm, and proj compute fills the early gaps.
            emit_smt_dma(0)
            emit_proj(0)
            for l in range(N_LAYERS):
                for b in range(nb):
                    emit_layer(b, l)
                    if l == 0 and b + 1 < nb:
                        emit_smt_dma(b + 1)
                        emit_proj(b + 1)
                    if l == N_LAYERS - 1:
                        emit_cls(b)

    nc.compile()
    return nc


def _route(x, router_w, router_b):
    logits = x.astype(np.float32) @ router_w.astype(np.float32).T + router_b
    p = np.exp(logits - logits.max(-1, keepdims=True))
    p /= p.sum(-1, keepdims=True)
    idx = np.argsort(-p, axis=-1, kind="stable")[:, :TOP_K]
    w = np.take_along_axis(p, idx, axis=-1)
    w = w / w.sum(-1, keepdims=True)
    return idx.astype(np.int64), w


def _kxm_layout(a, ktiles):
    """[K, M] (row-major, K=ktiles*128 rows) -> [128, ktiles, M] p-major."""
    k, m_dim = a.shape
    assert k == ktiles * 128
    return np.ascontiguousarray(a.reshape(ktiles, 128, m_dim).transpose(1, 0, 2))


def _run(inputs, trace=False):
    x = np.asarray(inputs["x"], np.float32)
    router_w = np.asarray(inputs["router_w"], np.float32)
    router_b = np.asarray(inputs["router_b"], np.float32)
    proj_w = np.asarray(inputs["proj_w"], np.float32)
    proj_b = np.asarray(inputs["proj_b"], np.float32)
    layer_w = np.asarray(inputs["layer_w"], np.float32)
    layer_b = np.asarray(inputs["layer_b"], np.float32)
    cls_w = np.asarray(inputs["cls_w"], np.float32)
    cls_b = np.asarray(inputs["cls_b"], np.float32)

    idx, w = _route(x, router_w, router_b)

    use_bias = bool(
        np.any(proj_b) or np.any(layer_b)
    )

    # blocks: (expert, token_rows, topk_col) chunks of <= C tokens
    blocks = []
    for e in range(N_EXPERTS):
        rows, cols = np.nonzero(idx == e)
        for s in range(0, len(rows), C):
            blocks.append((e, rows[s:s + C], cols[s:s + C]))
    nb = (len(blocks) + N_CORES - 1) // N_CORES

    key = (nb, use_bias)
    if key not in _PROGRAM_CACHE:
        _PROGRAM_CACHE[key] = _build_program(nb, use_bias)
    nc = _PROGRAM_CACHE[key]

    in_maps = [
        {
            "sm": np.zeros((nb, 128, SMW), BF16),
            "pb": np.zeros((nb, 128, MT), np.float32),
            "lw": np.zeros((nb, N_LAYERS, 2, 128, MT, 512), BF16),
            "lb": np.zeros((nb, 128, N_LAYERS, MT), np.float32),
        }
        for _ in range(N_CORES)
    ]

    placement = []  # (core, slot) per block
    for i, (e, rows, cols) in enumerate(blocks):
        core, slot = i % N_CORES, i // N_CORES
        placement.append((core, slot))
        m = in_maps[core]

        sm = m["sm"][slot]
        xpad = np.zeros((KP * 128, C), np.float32)
        xpad[:IN_DIM, :len(rows)] = x[rows].T
        sm[:, SM_X:SM_PW] = (
            xpad.reshape(KP, 128, C).transpose(1, 0, 2).reshape(128, KP * C)
        ).astype(BF16)
        pwp = np.zeros((KP * 128, HID), np.float32)
        pwp[:IN_DIM] = proj_w[e].T
        sm[:, SM_PW:SM_CW] = (
            pwp.reshape(KP, 128, HID).transpose(1, 0, 2).reshape(128, KP * HID)
        ).astype(BF16)
        cwp = np.zeros((HID, CLS_PAD), np.float32)
        cwp[:, :N_CLASSES] = cls_w[e].T
        sm[:, SM_CW:SMW] = (
            cwp.reshape(MT, 128, CLS_PAD).transpose(1, 0, 2).reshape(128, MT * CLS_PAD)
        ).astype(BF16)

        m["pb"][slot] = proj_b[e].reshape(MT, 128).T
        for l in range(N_LAYERS):
            # lhsT [1024(k*p), 1024(half*512+mcol)] -> [half, p, k, 512]
            m["lw"][slot, l] = (
                layer_w[e, l].T.reshape(MT, 128, 2, 512)
                .transpose(2, 1, 0, 3).astype(BF16)
            )
        m["lb"][slot] = layer_b[e].reshape(N_LAYERS, MT, 128).transpose(2, 0, 1)

    res = run_bass_kernel_spmd(
        nc, in_maps, core_ids=list(range(N_CORES)), trace=trace
    )

    out = np.zeros((BATCH, N_CLASSES), np.float32)
    for (e, rows, cols), (core, slot) in zip(blocks, placement):
        o = res.results[core]["out"][slot][:N_CLASSES, :len(rows)].T + cls_b[e]
        out[rows] += w[rows, cols, None] * o
    return out, res


def kernel(**inputs):
    out, _ = _run(inputs)
    return out


# revision 30
# speedup vs baseline: 1.0574x; 1.0574x over previous
"""Expert-parallel MoE classifier kernel for 8 Trainium2 NeuronCores.

Strategy
--------
The reference runs every expert densely over the whole batch, but the output
only uses each token's top-2 experts.  We therefore:

  1. (host) run the tiny router in fp32 numpy: logits -> softmax -> top-2 ->
     renormalized combine weights.  This exactly reproduces the jax routing
     (verified: identical top-2 indices, weight diff ~2e-6).
  2. (host) dispatch: gather each expert's assigned tokens into 128-token
     blocks; assign blocks round-robin to the 8 cores (expert-parallel
     sharding of proj_w/layer_w/cls_w: each expert's weights go only to the
     core that owns its block(s)).
  3. (device, SPMD on 8 cores) each core runs its NB blocks through the
     expert MLP stack: proj(267->1024) + gelu, 4x hidden(1024->1024) + gelu,
     cls(1024->5).  bf16 matmuls with fp32 PSUM accumulation; weights are
     the stationary operand, tokens ride the moving free dim.
  4. (host) combine: out[b] = w0*o_{e0}[b] + w1*o_{e1}[b] (+ cls bias).

Everything heavy (99.9% of FLOPs) runs on the NeuronCores.
"""

import sys
import types

import numpy as np
import ml_dtypes

import concourse.bacc as bacc
import concourse.tile as tile
import concourse.mybir as mybir
from concourse.bass_utils import run_bass_kernel_spmd


def _ensure_ntff_hook():
    """Provide antenv.axon_hooks if the image's antenv lacks it, so
    trace=True NTFF profiling works under axon (see trn_agent_boot)."""
    try:
        from antenv.axon_hooks import get_axon_ntff_profile_hook  # noqa: F401
        return
    except ImportError:
        pass
    try:
        from trn_agent_boot.trn_boot import _ntff_profile_via_ctypes
        hook = _ntff_profile_via_ctypes("/opt/axon/libaxon_pjrt.so")
    except Exception:
        hook = None
    mod = types.ModuleType("antenv.axon_hooks")
    mod.get_axon_ntff_profile_hook = lambda: hook
    mod.set_axon_ntff_profile_hook = lambda h: None
    sys.modules["antenv.axon_hooks"] = mod


_ensure_ntff_hook()

# Model shape (hardcoded per problem spec)
N_EXPERTS = 23
IN_DIM = 267
HID = 1024
N_LAYERS = 4
N_CLASSES = 5
TOP_K = 2
BATCH = 1024
N_CORES = 8

C = 144          # tokens per block (moving free dim of every matmul).
                 # Sized so PE compute per layer (~64 matmuls of N=C) fills
                 # the ~4.8us layer weight-DMA period: the PE never idles
                 # long enough to trip the HAM clock throttle. The extra
                 # token padding is free -- the DMA stream is the floor.
KP = 3           # K-tiles for the 267-dim input contraction (padded to 384)
MT = HID // 128  # 8 m-tiles (output partition tiles) per 1024-wide layer
MG = MT // 2     # m-tile pairs: two m-tiles share one PSUM bank / one ACT
CLS_PAD = 8      # classes padded 5 -> 8

BF16 = ml_dtypes.bfloat16

_PROGRAM_CACHE = {}


# merged per-block "smalls" tensor column offsets (bf16 columns per partition).
# Full 128-row K-tiles (k=0,1) of x.T and proj_w.T plus cls_w.T; the 11-row
# k=2 tail (input rows 256..266) rides in a separate small [11, C+HID] DMA.
KT = IN_DIM - 256  # 11
SM_X = 0
SM_PW = 2 * C
SM_CW = SM_PW + 2 * HID
SMW = SM_CW + MT * CLS_PAD


def _build_program(nb, use_bias):
    """Bass/Tile program for one core: nb expert blocks through the MLP."""
    dt = mybir.dt
    nc = bacc.Bacc("TRN2", target_bir_lowering=False, debug=False)

    sm_d = nc.dram_tensor("sm", [nb, 128, SMW], dt.bfloat16, kind="ExternalInput").ap()
    smtail_d = nc.dram_tensor("smtail", [nb, KT, C + HID], dt.bfloat16, kind="ExternalInput").ap()
    pb_d = nc.dram_tensor("pb", [nb, 128, MT], dt.float32, kind="ExternalInput").ap()
    # layer weights: [l, half, p, k, 512] (half = output-column half)
    lw_d = nc.dram_tensor("lw", [nb, N_LAYERS, 2, 128, MT, 512], dt.bfloat16, kind="ExternalInput").ap()
    lb_d = nc.dram_tensor("lb", [nb, 128, N_LAYERS, MT], dt.float32, kind="ExternalInput").ap()
    out_d = nc.dram_tensor("out", [nb, CLS_PAD, C], dt.float32, kind="ExternalOutput").ap()

    gelu = mybir.ActivationFunctionType.Gelu_apprx_tanh

    def act(hm, ps, bias_slices):
        # hm/ps: [128, 2, C]; bias_slices: list of (half, bias_ap[128,1]) or falsy
        if use_bias:
            for h2, b_ap in bias_slices:
                nc.scalar.activation(hm[:, h2, :], ps[:, h2, :], gelu, bias=b_ap)
        else:
            nc.scalar.activation(hm[:, :, :], ps[:, :, :], gelu)

    with tile.TileContext(nc) as tc:
        with (
            tc.tile_pool(name="wpool", bufs=9) as wpool,
            tc.tile_pool(name="small", bufs=max(3, nb)) as small,
            tc.tile_pool(name="hpool", bufs=4 * nb + 8) as hpool,
            tc.tile_pool(name="opool", bufs=2) as opool,
            tc.tile_pool(name="pspool", bufs=6, space="PSUM") as pspool,
            tc.tile_pool(name="cpspool", bufs=2, space="PSUM") as cpspool,
        ):
            smts = [None] * nb
            biases = [None] * nb
            hs = [None] * nb

            def emit_smt_dma(b):
                smt = small.tile([128, SMW], dt.bfloat16)
                nc.sync.dma_start(out=smt, in_=sm_d[b])
                smtail = small.tile([KT, C + HID], dt.bfloat16)
                nc.sync.dma_start(out=smtail, in_=smtail_d[b])
                smts[b] = (smt, smtail)
                if use_bias:
                    pbt = small.tile([128, MT], dt.float32)
                    nc.sync.dma_start(out=pbt, in_=pb_d[b])
                    lbt = small.tile([128, N_LAYERS, MT], dt.float32)
                    nc.sync.dma_start(out=lbt, in_=lb_d[b])
                    biases[b] = (pbt, lbt)

            def emit_proj(b):
                smt, smtail = smts[b]
                h = []
                for mg in range(MG):
                    ps = pspool.tile([128, 2, C], dt.float32)
                    for h2 in range(2):
                        m = 2 * mg + h2
                        for k in range(2):
                            nc.tensor.matmul(
                                ps[:, h2, :],
                                smt[:, SM_PW + k * HID + m * 128: SM_PW + k * HID + (m + 1) * 128],
                                smt[:, SM_X + k * C: SM_X + (k + 1) * C],
                                start=(k == 0), stop=False,
                            )
                        nc.tensor.matmul(
                            ps[:, h2, :],
                            smtail[:, C + m * 128: C + (m + 1) * 128],
                            smtail[:, 0:C],
                            start=False, stop=True,
                        )
                    hm = hpool.tile([128, 2, C], dt.bfloat16)
                    act(hm, ps, use_bias and [(h2, biases[b][0][:, 2 * mg + h2:2 * mg + h2 + 1]) for h2 in range(2)])
                    h.append(hm)
                hs[b] = h

            def emit_layer(b, l):
                h = hs[b]
                lwt = wpool.tile([128, 2, MT, 512], dt.bfloat16)
                if b == nb - 1 and l == N_LAYERS - 1:
                    # final chunk of the stream in halves: tail compute
                    # overlaps the end of the stream
                    nc.sync.dma_start(out=lwt[:, 0], in_=lw_d[b, l, 0])
                    nc.sync.dma_start(out=lwt[:, 1], in_=lw_d[b, l, 1])
                else:
                    nc.sync.dma_start(
                        out=lwt, in_=lw_d[b, l].rearrange("two p k m -> p two k m")
                    )
                hn = []
                for mg in range(MG):
                    ps = pspool.tile([128, 2, C], dt.float32)
                    for h2 in range(2):
                        m = 2 * mg + h2
                        mcol = (m % 4) * 128
                        for k in range(MT):
                            nc.tensor.matmul(
                                ps[:, h2, :], lwt[:, m // 4, k, mcol:mcol + 128],
                                h[k // 2][:, k % 2, :],
                                start=(k == 0), stop=(k == MT - 1),
                            )
                    hm = hpool.tile([128, 2, C], dt.bfloat16)
                    act(hm, ps, use_bias and [(h2, biases[b][1][:, l, 2 * mg + h2:2 * mg + h2 + 1]) for h2 in range(2)])
                    hn.append(hm)
                hs[b] = hn

            def emit_cls(b):
                (smt, _), h = smts[b], hs[b]
                cps = cpspool.tile([CLS_PAD, C], dt.float32)
                for k in range(MT):
                    nc.tensor.matmul(
                        cps[:], smt[:, SM_CW + k * CLS_PAD: SM_CW + (k + 1) * CLS_PAD],
                        h[k // 2][:, k % 2, :],
                        start=(k == 0), stop=(k == MT - 1),
                    )
                ot = opool.tile([CLS_PAD, C], dt.float32)
                nc.vector.tensor_copy(ot[:], cps[:])
                # output DMA rides the ACT HWDGE ring (tiny, and by dispatch
                # time its data is ready, so it doesn't stall weight DMAs)
                nc.scalar.dma_start(out=out_d[b], in_=ot[:])

            # Interleave the blocks layer-by-layer (A0,B0,C0,A1,B1,C1,...):
            # each layer's weight wait shrinks below the HAM idle window, so
            # the PE stays warm, and proj compute fills the early gaps.
            emit_smt_dma(0)
            emit_proj(0)
            for l in range(N_LAYERS):
                for b in range(nb):
                    emit_layer(b, l)
                    if l == 0 and b + 1 < nb:
                        emit_smt_dma(b + 1)
                        emit_proj(b + 1)
                    if l == N_LAYERS - 1:
                        emit_cls(b)

    nc.compile()
    return nc


def _route(x, router_w, router_b):
    logits = x.astype(np.float32) @ router_w.astype(np.float32).T + router_b
    p = np.exp(logits - logits.max(-1, keepdims=True))
    p /= p.sum(-1, keepdims=True)
    idx = np.argsort(-p, axis=-1, kind="stable")[:, :TOP_K]
    w = np.take_along_axis(p, idx, axis=-1)
    w = w / w.sum(-1, keepdims=True)
    return idx.astype(np.int64), w


def _kxm_layout(a, ktiles):
    """[K, M] (row-major, K=ktiles*128 rows) -> [128, ktiles, M] p-major."""
    k, m_dim = a.shape
    assert k == ktiles * 128
    return np.ascontiguousarray(a.reshape(ktiles, 128, m_dim).transpose(1, 0, 2))


def _run(inputs, trace=False):
    x = np.asarray(inputs["x"], np.float32)
    router_w = np.asarray(inputs["router_w"], np.float32)
    router_b = np.asarray(inputs["router_b"], np.float32)
    proj_w = np.asarray(inputs["proj_w"], np.float32)
    proj_b = np.asarray(inputs["proj_b"], np.float32)
    layer_w = np.asarray(inputs["layer_w"], np.float32)
    layer_b = np.asarray(inputs["layer_b"], np.float32)
    cls_w = np.asarray(inputs["cls_w"], np.float32)
    cls_b = np.asarray(inputs["cls_b"], np.float32)

    idx, w = _route(x, router_w, router_b)

    use_bias = bool(
        np.any(proj_b) or np.any(layer_b)
    )

    # blocks: (expert, token_rows, topk_col) chunks of <= C tokens
    blocks = []
    for e in range(N_EXPERTS):
        rows, cols = np.nonzero(idx == e)
        for s in range(0, len(rows), C):
            blocks.append((e, rows[s:s + C], cols[s:s + C]))
    nb = (len(blocks) + N_CORES - 1) // N_CORES

    key = (nb, use_bias)
    if key not in _PROGRAM_CACHE:
        _PROGRAM_CACHE[key] = _build_program(nb, use_bias)
    nc = _PROGRAM_CACHE[key]

    in_maps = [
        {
            "sm": np.zeros((nb, 128, SMW), BF16),
            "smtail": np.zeros((nb, KT, C + HID), BF16),
            "pb": np.zeros((nb, 128, MT), np.float32),
            "lw": np.zeros((nb, N_LAYERS, 2, 128, MT, 512), BF16),
            "lb": np.zeros((nb, 128, N_LAYERS, MT), np.float32),
        }
        for _ in range(N_CORES)
    ]

    placement = []  # (core, slot) per block
    for i, (e, rows, cols) in enumerate(blocks):
        core, slot = i % N_CORES, i // N_CORES
        placement.append((core, slot))
        m = in_maps[core]

        sm = m["sm"][slot]
        xb = np.zeros((IN_DIM, C), np.float32)
        xb[:, :len(rows)] = x[rows].T
        sm[:, SM_X:SM_PW] = (
            xb[:256].reshape(2, 128, C).transpose(1, 0, 2).reshape(128, 2 * C)
        ).astype(BF16)
        pwt = proj_w[e].T  # [267, 1024]
        sm[:, SM_PW:SM_CW] = (
            pwt[:256].reshape(2, 128, HID).transpose(1, 0, 2).reshape(128, 2 * HID)
        ).astype(BF16)
        m["smtail"][slot, :, :C] = xb[256:].astype(BF16)
        m["smtail"][slot, :, C:] = pwt[256:].astype(BF16)
        cwp = np.zeros((HID, CLS_PAD), np.float32)
        cwp[:, :N_CLASSES] = cls_w[e].T
        sm[:, SM_CW:SMW] = (
            cwp.reshape(MT, 128, CLS_PAD).transpose(1, 0, 2).reshape(128, MT * CLS_PAD)
        ).astype(BF16)

        m["pb"][slot] = proj_b[e].reshape(MT, 128).T
        for l in range(N_LAYERS):
            # lhsT [1024(k*p), 1024(half*512+mcol)] -> [half, p, k, 512]
            m["lw"][slot, l] = (
                layer_w[e, l].T.reshape(MT, 128, 2, 512)
                .transpose(2, 1, 0, 3).astype(BF16)
            )
        m["lb"][slot] = layer_b[e].reshape(N_LAYERS, MT, 128).transpose(2, 0, 1)

    res = run_bass_kernel_spmd(
        nc, in_maps, core_ids=list(range(N_CORES)), trace=trace
    )

    out = np.zeros((BATCH, N_CLASSES), np.float32)
    for (e, rows, cols), (core, slot) in zip(blocks, placement):
        o = res.results[core]["out"][slot][:N_CLASSES, :len(rows)].T + cls_b[e]
        out[rows] += w[rows, cols, None] * o
    return out, res


def kernel(**inputs):
    out, _ = _run(inputs)
    return out


# revision 34
# speedup vs baseline: 1.0691x; 1.0111x over previous
"""Expert-parallel MoE classifier kernel for 8 Trainium2 NeuronCores.

Strategy
--------
The reference runs every expert densely over the whole batch, but the output
only uses each token's top-2 experts.  We therefore:

  1. (host) run the tiny router in fp32 numpy: logits -> softmax -> top-2 ->
     renormalized combine weights.  This exactly reproduces the jax routing
     (verified: identical top-2 indices, weight diff ~2e-6).
  2. (host) dispatch: gather each expert's assigned tokens into 128-token
     blocks; assign blocks round-robin to the 8 cores (expert-parallel
     sharding of proj_w/layer_w/cls_w: each expert's weights go only to the
     core that owns its block(s)).
  3. (device, SPMD on 8 cores) each core runs its NB blocks through the
     expert MLP stack: proj(267->1024) + gelu, 4x hidden(1024->1024) + gelu,
     cls(1024->5).  bf16 matmuls with fp32 PSUM accumulation; weights are
     the stationary operand, tokens ride the moving free dim.
  4. (host) combine: out[b] = w0*o_{e0}[b] + w1*o_{e1}[b] (+ cls bias).

Everything heavy (99.9% of FLOPs) runs on the NeuronCores.
"""

import sys
import types

import numpy as np
import ml_dtypes

import concourse.bacc as bacc
import concourse.tile as tile
import concourse.mybir as mybir
from concourse.bass_utils import run_bass_kernel_spmd


def _ensure_ntff_hook():
    """Provide antenv.axon_hooks if the image's antenv lacks it, so
    trace=True NTFF profiling works under axon (see trn_agent_boot)."""
    try:
        from antenv.axon_hooks import get_axon_ntff_profile_hook  # noqa: F401
        return
    except ImportError:
        pass
    try:
        from trn_agent_boot.trn_boot import _ntff_profile_via_ctypes
        hook = _ntff_profile_via_ctypes("/opt/axon/libaxon_pjrt.so")
    except Exception:
        hook = None
    mod = types.ModuleType("antenv.axon_hooks")
    mod.get_axon_ntff_profile_hook = lambda: hook
    mod.set_axon_ntff_profile_hook = lambda h: None
    sys.modules["antenv.axon_hooks"] = mod


_ensure_ntff_hook()

# Model shape (hardcoded per problem spec)
N_EXPERTS = 23
IN_DIM = 267
HID = 1024
N_LAYERS = 4
N_CLASSES = 5
TOP_K = 2
BATCH = 1024
N_CORES = 8

C = 144          # tokens per block (moving free dim of every matmul).
                 # Sized so PE compute per layer (~64 matmuls of N=C) fills
                 # the ~4.8us layer weight-DMA period: the PE never idles
                 # long enough to trip the HAM clock throttle. The extra
                 # token padding is free -- the DMA stream is the floor.
KP = 3           # K-tiles for the 267-dim input contraction (padded to 384)
MT = HID // 128  # 8 m-tiles (output partition tiles) per 1024-wide layer
MG = MT // 2     # m-tile pairs: two m-tiles share one PSUM bank / one ACT
CLS_PAD = 8      # classes padded 5 -> 8

BF16 = ml_dtypes.bfloat16

_PROGRAM_CACHE = {}


# merged per-block "smalls" tensor column offsets (bf16 columns per partition).
# Full 128-row K-tiles (k=0,1) of x.T and proj_w.T plus cls_w.T; the 11-row
# k=2 tail (input rows 256..266) rides in a separate small [11, C+HID] DMA.
KT = IN_DIM - 256  # 11
SM_X = 0
SM_PW = 2 * C
SM_CW = SM_PW + 2 * HID
SMW = SM_CW + MT * CLS_PAD


def _build_program(nb, use_bias):
    """Bass/Tile program for one core: nb expert blocks through the MLP."""
    dt = mybir.dt
    nc = bacc.Bacc("TRN2", target_bir_lowering=False, debug=False)

    sm_d = nc.dram_tensor("sm", [nb, 128, SMW], dt.bfloat16, kind="ExternalInput").ap()
    smtail_d = nc.dram_tensor("smtail", [nb, KT, C + HID], dt.bfloat16, kind="ExternalInput").ap()
    pb_d = nc.dram_tensor("pb", [nb, 128, MT], dt.float32, kind="ExternalInput").ap()
    # layer weights: [l, half, p, k, 512] (half = output-column half)
    lw_d = nc.dram_tensor("lw", [nb, N_LAYERS, 2, 128, MT, 512], dt.bfloat16, kind="ExternalInput").ap()
    lb_d = nc.dram_tensor("lb", [nb, 128, N_LAYERS, MT], dt.float32, kind="ExternalInput").ap()
    out_d = nc.dram_tensor("out", [nb, CLS_PAD, C], dt.float32, kind="ExternalOutput").ap()

    gelu = mybir.ActivationFunctionType.Gelu_apprx_tanh

    def act(hm, ps, bias_slices):
        # hm/ps: [128, 2, C]; two half-ACTs so each h half releases to the
        # next layer's matmuls as soon as it's ready
        if use_bias:
            for h2, b_ap in bias_slices:
                nc.scalar.activation(hm[:, h2, :], ps[:, h2, :], gelu, bias=b_ap)
        else:
            for h2 in range(2):
                nc.scalar.activation(hm[:, h2, :], ps[:, h2, :], gelu)

    with tile.TileContext(nc, pool_alloc_mode="queue") as tc:
        with (
            tc.tile_pool(name="wpool", bufs=9) as wpool,
            tc.tile_pool(name="small", bufs=max(3, nb)) as small,
            tc.tile_pool(name="hpool", bufs=4 * nb + 8) as hpool,
            tc.tile_pool(name="opool", bufs=2) as opool,
            tc.tile_pool(name="pspool", bufs=6, space="PSUM") as pspool,
            tc.tile_pool(name="cpspool", bufs=2, space="PSUM") as cpspool,
        ):
            smts = [None] * nb
            biases = [None] * nb
            hs = [None] * nb

            def emit_smt_dma(b):
                smt = small.tile([128, SMW], dt.bfloat16)
                nc.sync.dma_start(out=smt, in_=sm_d[b])
                smtail = small.tile([KT, C + HID], dt.bfloat16)
                nc.sync.dma_start(out=smtail, in_=smtail_d[b])
                smts[b] = (smt, smtail)
                if use_bias:
                    pbt = small.tile([128, MT], dt.float32)
                    nc.sync.dma_start(out=pbt, in_=pb_d[b])
                    lbt = small.tile([128, N_LAYERS, MT], dt.float32)
                    nc.sync.dma_start(out=lbt, in_=lb_d[b])
                    biases[b] = (pbt, lbt)

            def emit_proj(b):
                smt, smtail = smts[b]
                h = []
                for mg in range(MG):
                    ps = pspool.tile([128, 2, C], dt.float32)
                    for h2 in range(2):
                        m = 2 * mg + h2
                        for k in range(2):
                            nc.tensor.matmul(
                                ps[:, h2, :],
                                smt[:, SM_PW + k * HID + m * 128: SM_PW + k * HID + (m + 1) * 128],
                                smt[:, SM_X + k * C: SM_X + (k + 1) * C],
                                start=(k == 0), stop=False,
                            )
                        nc.tensor.matmul(
                            ps[:, h2, :],
                            smtail[:, C + m * 128: C + (m + 1) * 128],
                            smtail[:, 0:C],
                            start=False, stop=True,
                        )
                    hm = hpool.tile([128, 2, C], dt.bfloat16)
                    act(hm, ps, use_bias and [(h2, biases[b][0][:, 2 * mg + h2:2 * mg + h2 + 1]) for h2 in range(2)])
                    h.append(hm)
                hs[b] = h

            def emit_layer(b, l):
                h = hs[b]
                lwt = wpool.tile([128, 2, MT, 512], dt.bfloat16)
                if b == nb - 1 and l == N_LAYERS - 1:
                    # final chunk of the stream in halves: tail compute
                    # overlaps the end of the stream
                    nc.sync.dma_start(out=lwt[:, 0], in_=lw_d[b, l, 0])
                    nc.sync.dma_start(out=lwt[:, 1], in_=lw_d[b, l, 1])
                else:
                    nc.sync.dma_start(
                        out=lwt, in_=lw_d[b, l].rearrange("two p k m -> p two k m")
                    )
                hn = []
                for mg in range(MG):
                    ps = pspool.tile([128, 2, C], dt.float32)
                    for h2 in range(2):
                        m = 2 * mg + h2
                        mcol = (m % 4) * 128
                        for k in range(MT):
                            nc.tensor.matmul(
                                ps[:, h2, :], lwt[:, m // 4, k, mcol:mcol + 128],
                                h[k // 2][:, k % 2, :],
                                start=(k == 0), stop=(k == MT - 1),
                            )
                    hm = hpool.tile([128, 2, C], dt.bfloat16)
                    act(hm, ps, use_bias and [(h2, biases[b][1][:, l, 2 * mg + h2:2 * mg + h2 + 1]) for h2 in range(2)])
                    hn.append(hm)
                hs[b] = hn

            def emit_cls(b):
                (smt, _), h = smts[b], hs[b]
                cps = cpspool.tile([CLS_PAD, C], dt.float32)
                for k in range(MT):
                    nc.tensor.matmul(
                        cps[:], smt[:, SM_CW + k * CLS_PAD: SM_CW + (k + 1) * CLS_PAD],
                        h[k // 2][:, k % 2, :],
                        start=(k == 0), stop=(k == MT - 1),
                    )
                ot = opool.tile([CLS_PAD, C], dt.float32)
                nc.vector.tensor_copy(ot[:], cps[:])
                # output DMA rides the ACT HWDGE ring (tiny, and by dispatch
                # time its data is ready, so it doesn't stall weight DMAs)
                nc.scalar.dma_start(out=out_d[b], in_=ot[:])

            # Interleave the blocks layer-by-layer (A0,B0,C0,A1,B1,C1,...):
            # each layer's weight wait shrinks below the HAM idle window, so
            # the PE stays warm, and proj compute fills the early gaps.
            emit_smt_dma(0)
            emit_proj(0)
            for l in range(N_LAYERS):
                for b in range(nb):
                    emit_layer(b, l)
                    if l == 0 and b + 1 < nb:
                        emit_smt_dma(b + 1)
                        emit_proj(b + 1)
                    if l == N_LAYERS - 1:
                        emit_cls(b)

    nc.compile()
    return nc


def _route(x, router_w, router_b):
    logits = x.astype(np.float32) @ router_w.astype(np.float32).T + router_b
    p = np.exp(logits - logits.max(-1, keepdims=True))
    p /= p.sum(-1, keepdims=True)
    idx = np.argsort(-p, axis=-1, kind="stable")[:, :TOP_K]
    w = np.take_along_axis(p, idx, axis=-1)
    w = w / w.sum(-1, keepdims=True)
    return idx.astype(np.int64), w


def _kxm_layout(a, ktiles):
    """[K, M] (row-major, K=ktiles*128 rows) -> [128, ktiles, M] p-major."""
    k, m_dim = a.shape
    assert k == ktiles * 128
    return np.ascontiguousarray(a.reshape(ktiles, 128, m_dim).transpose(1, 0, 2))


def _run(inputs, trace=False):
    x = np.asarray(inputs["x"], np.float32)
    router_w = np.asarray(inputs["router_w"], np.float32)
    router_b = np.asarray(inputs["router_b"], np.float32)
    proj_w = np.asarray(inputs["proj_w"], np.float32)
    proj_b = np.asarray(inputs["proj_b"], np.float32)
    layer_w = np.asarray(inputs["layer_w"], np.float32)
    layer_b = np.asarray(inputs["layer_b"], np.float32)
    cls_w = np.asarray(inputs["cls_w"], np.float32)
    cls_b = np.asarray(inputs["cls_b"], np.float32)

    idx, w = _route(x, router_w, router_b)

    use_bias = bool(
        np.any(proj_b) or np.any(layer_b)
    )

    # blocks: (expert, token_rows, topk_col) chunks of <= C tokens
    blocks = []
    for e in range(N_EXPERTS):
        rows, cols = np.nonzero(idx == e)
        for s in range(0, len(rows), C):
            blocks.append((e, rows[s:s + C], cols[s:s + C]))
    nb = (len(blocks) + N_CORES - 1) // N_CORES

    key = (nb, use_bias)
    if key not in _PROGRAM_CACHE:
        _PROGRAM_CACHE[key] = _build_program(nb, use_bias)
    nc = _PROGRAM_CACHE[key]

    in_maps = [
        {
            "sm": np.zeros((nb, 128, SMW), BF16),
            "smtail": np.zeros((nb, KT, C + HID), BF16),
            "pb": np.zeros((nb, 128, MT), np.float32),
            "lw": np.zeros((nb, N_LAYERS, 2, 128, MT, 512), BF16),
            "lb": np.zeros((nb, 128, N_LAYERS, MT), np.float32),
        }
        for _ in range(N_CORES)
    ]

    placement = []  # (core, slot) per block
    for i, (e, rows, cols) in enumerate(blocks):
        core, slot = i % N_CORES, i // N_CORES
        placement.append((core, slot))
        m = in_maps[core]

        sm = m["sm"][slot]
        xb = np.zeros((IN_DIM, C), np.float32)
        xb[:, :len(rows)] = x[rows].T
        sm[:, SM_X:SM_PW] = (
            xb[:256].reshape(2, 128, C).transpose(1, 0, 2).reshape(128, 2 * C)
        ).astype(BF16)
        pwt = proj_w[e].T  # [267, 1024]
        sm[:, SM_PW:SM_CW] = (
            pwt[:256].reshape(2, 128, HID).transpose(1, 0, 2).reshape(128, 2 * HID)
        ).astype(BF16)
        m["smtail"][slot, :, :C] = xb[256:].astype(BF16)
        m["smtail"][slot, :, C:] = pwt[256:].astype(BF16)
        cwp = np.zeros((HID, CLS_PAD), np.float32)
        cwp[:, :N_CLASSES] = cls_w[e].T
        sm[:, SM_CW:SMW] = (
            cwp.reshape(MT, 128, CLS_PAD).transpose(1, 0, 2).reshape(128, MT * CLS_PAD)
        ).astype(BF16)

        m["pb"][slot] = proj_b[e].reshape(MT, 128).T
        for l in range(N_LAYERS):
            # lhsT [1024(k*p), 1024(half*512+mcol)] -> [half, p, k, 512]
            m["lw"][slot, l] = (
                layer_w[e, l].T.reshape(MT, 128, 2, 512)
                .transpose(2, 1, 0, 3).astype(BF16)
            )
        m["lb"][slot] = layer_b[e].reshape(N_LAYERS, MT, 128).transpose(2, 0, 1)

    res = run_bass_kernel_spmd(
        nc, in_maps, core_ids=list(range(N_CORES)), trace=trace
    )

    out = np.zeros((BATCH, N_CLASSES), np.float32)
    for (e, rows, cols), (core, slot) in zip(blocks, placement):
        o = res.results[core]["out"][slot][:N_CLASSES, :len(rows)].T + cls_b[e]
        out[rows] += w[rows, cols, None] * o
    return out, res


def kernel(**inputs):
    out, _ = _run(inputs)
    return out
